# revision 3
# baseline (speedup 1.0000x reference)
"""HMM forward-algorithm loss on 8 NeuronCores (Bass/Tile) — v3.

v3 vs v2 (trace-driven):
 - bc broadcast matmuls in bf16 (were fp32 two-pass: ~2.6us PE per tile).
 - renorm: snapshot at local step j==2, factor applied to the SAME tile's
   last e-slice (cols 120:128) -> no cross-tile rq dependency stalls.
 - compact per-core table: only the ~14K vocab rows this core's chains
   touch (unique of its 8 batch rows' x), padded to 16384 (= 8 clean
   [128,2048] chunks); row 16384 holds s/128 for the "ones" slots.
 - s[k] (softmax normalizer over the FULL vocab) from a per-core V/8
   shard sweep + AllReduce[1,128] (USE_COLLECTIVE), else full sweep.
 - build fused in groups of 4 tiles: [128,512] PSUM L-tile, one ACT exp,
   one 512-col s-matmul, one [512,128] table write.
"""

import numpy as np

N, T, K, V = 32, 4096, 128, 50000
P = 128
HALF = T // 2
R = 16
C0 = 40.0
NSLOT = HALF + 1
NROW = 8
NTILE_G = NSLOT * NROW // P      # 128 full gather tiles
REM = NSLOT * NROW - NTILE_G * P # 8
UNI = 16384                      # compact table rows (padded)
SH_ROWS = V // 8                 # 6250 vocab rows per core for the s sweep
SH_TILES = 49                    # padded to 49*128 = 6272

USE_COLLECTIVE = True

_CACHE = {}


def _build_nc():
    import concourse.bass as bass
    import concourse.mybir as mybir
    import concourse.tile as tile
    from concourse import bacc
    from concourse.masks import make_identity

    f32 = mybir.dt.float32
    bf16 = mybir.dt.bfloat16
    i32 = mybir.dt.int32
    EXP = mybir.ActivationFunctionType.Exp
    LN = mybir.ActivationFunctionType.Ln
    COPY = mybir.ActivationFunctionType.Copy
    SUB = mybir.AluOpType.subtract

    nc = bacc.Bacc("TRN2", target_bir_lowering=False, debug=False, num_devices=8)

    tr_d = nc.dram_tensor("tr", [K, K], f32, kind="ExternalInput")
    emb_d = nc.dram_tensor("emb", [K, K], f32, kind="ExternalInput")
    vocs_d = nc.dram_tensor("vocs", [K, UNI], bf16, kind="ExternalInput")
    vocsh_d = nc.dram_tensor("vocsh", [K, SH_TILES * P], bf16, kind="ExternalInput")
    sw_d = nc.dram_tensor("sw", [K, 1], f32, kind="ExternalInput")
    sb_d = nc.dram_tensor("sb", [K, 1], f32, kind="ExternalInput")
    dirf_d = nc.dram_tensor("dirf", [K, 1], f32, kind="ExternalInput")
    offs_d = nc.dram_tensor("offs", [P, NTILE_G + 1], i32, kind="ExternalInput")

    outp_d = nc.dram_tensor("outp", [K, NROW], f32, kind="ExternalOutput")
    accr_d = nc.dram_tensor("accr", [1, NROW], f32, kind="ExternalOutput")
    lnacc_d = nc.dram_tensor("lnacc", [1, P], f32, kind="ExternalOutput")

    table_d = nc.dram_tensor("table2", [UNI + 1, K], bf16)
    sP_d = nc.dram_tensor("sP", [1, K], f32)
    sG_d = nc.dram_tensor("sG", [1, K], f32)

    with tile.TileContext(nc) as tc:
        with (
            tc.tile_pool(name="csb", bufs=1) as csb,
            tc.tile_pool(name="sbA", bufs=3) as sbA,
            tc.tile_pool(name="sbB", bufs=2) as sbB,
            tc.tile_pool(name="sbS", bufs=3) as sbS,
            tc.tile_pool(name="p_t", bufs=2, space="PSUM") as p_t,
            tc.tile_pool(name="p_m", bufs=2, space="PSUM") as p_m,
            tc.tile_pool(name="p_cs", bufs=2, space="PSUM") as p_cs,
            tc.tile_pool(name="p_bc", bufs=2, space="PSUM") as p_bc,
        ):
            # ---------- constants ----------
            ident = csb.tile([P, P], dtype=f32)
            make_identity(nc, ident[:])
            identb = csb.tile([P, P], dtype=bf16)
            nc.vector.tensor_copy(out=identb[:], in_=ident[:])
            onesb_col = csb.tile([P, 1], dtype=bf16)
            nc.vector.memset(onesb_col[:], 1.0)
            ones128_row = csb.tile([1, P], dtype=bf16)
            nc.vector.memset(ones128_row[:], 128.0)
            negc0 = csb.tile([P, 1], dtype=f32)
            nc.vector.memset(negc0[:], -C0)

            trt = csb.tile([P, P], dtype=f32)
            nc.sync.dma_start(out=trt[:], in_=tr_d[:, :])
            embi = csb.tile([P, P], dtype=f32)
            nc.sync.dma_start(out=embi[:], in_=emb_d[:, :])
            swt = csb.tile([P, 1], dtype=f32)
            nc.sync.dma_start(out=swt[:], in_=sw_d[:, :])
            sbt = csb.tile([P, 1], dtype=f32)
            nc.sync.dma_start(out=sbt[:], in_=sb_d[:, :])
            dirt = csb.tile([P, 1], dtype=f32)
            nc.sync.dma_start(out=dirt[:], in_=dirf_d[:, :])
            offs_sb = csb.tile([P, NTILE_G + 1], dtype=i32)
            nc.sync.dma_start(out=offs_sb[:], in_=offs_d[:, :])

            tp0 = p_t.tile([P, P], dtype=f32, tag="pt")
            nc.tensor.transpose(out=tp0[:], in_=embi[:], identity=ident[:])
            embTb = csb.tile([P, P], dtype=bf16)
            nc.scalar.copy(out=embTb[:], in_=tp0[:])

            rm = csb.tile([P, 1], dtype=f32)
            nc.vector.tensor_reduce(
                out=rm[:], in_=trt[:], axis=mybir.AxisListType.X, op=mybir.AluOpType.max
            )
            nrm = csb.tile([P, 1], dtype=f32)
            nc.vector.tensor_scalar_mul(out=nrm[:], in0=rm[:], scalar1=-1.0)
            rs = csb.tile([P, 1], dtype=f32)
            eLt = csb.tile([P, P], dtype=f32)
            nc.scalar.activation(
                out=eLt[:], in_=trt[:], func=EXP, bias=nrm[:, :1], accum_out=rs[:, :1]
            )
            rrs = csb.tile([P, 1], dtype=f32)
            nc.vector.reciprocal(out=rrs[:], in_=rs[:])
            Texp = csb.tile([P, P], dtype=f32)
            nc.vector.tensor_scalar_mul(out=Texp[:], in0=eLt[:], scalar1=rrs[:, :1])
            tp1 = p_t.tile([P, P], dtype=f32, tag="pt")
            nc.tensor.transpose(out=tp1[:], in_=Texp[:], identity=ident[:])
            TexpT = csb.tile([P, P], dtype=f32)
            nc.scalar.copy(out=TexpT[:], in_=tp1[:])
            dif = csb.tile([P, P], dtype=f32)
            nc.vector.tensor_tensor(out=dif[:], in0=Texp[:], in1=TexpT[:], op=SUB)
            dif2 = csb.tile([P, P], dtype=f32)
            nc.vector.tensor_scalar_mul(out=dif2[:], in0=dif[:], scalar1=dirt[:, :1])
            Wf = csb.tile([P, P], dtype=f32)
            nc.vector.tensor_add(out=Wf[:], in0=dif2[:], in1=TexpT[:])
            Wb = csb.tile([P, P], dtype=bf16)
            nc.vector.tensor_copy(out=Wb[:], in_=Wf[:])

            p0s = csb.tile([P, 1], dtype=f32)
            nc.vector.tensor_add(out=p0s[:], in0=swt[:], in1=sbt[:])
            p0e = csb.tile([P, 1], dtype=f32)
            nc.scalar.activation(out=p0e[:], in_=p0s[:], func=EXP)
            t1 = csb.tile([P, 1], dtype=f32)
            nc.vector.tensor_scalar_add(out=t1[:], in0=p0e[:], scalar1=-1.0)
            t2 = csb.tile([P, 1], dtype=f32)
            nc.vector.tensor_mul(out=t2[:], in0=t1[:], in1=dirt[:])
            initc = csb.tile([P, 1], dtype=f32)
            nc.vector.tensor_scalar_add(out=initc[:], in0=t2[:], scalar1=1.0)
            ones8 = csb.tile([P, NROW], dtype=f32)
            nc.vector.memset(ones8[:], 1.0)
            q0 = csb.tile([P, NROW], dtype=f32)
            nc.vector.tensor_scalar_mul(out=q0[:], in0=ones8[:], scalar1=initc[:, :1])

            lnacc_sb = csb.tile([1, P], dtype=f32)
            nc.vector.memset(lnacc_sb[:], 0.0)
            accr_sb = csb.tile([1, NROW], dtype=f32)
            nc.vector.memset(accr_sb[:], 0.0)

            # ---------- pass A: transpose-free sweeps (lhsT = host-transposed bf16) ----------
            vocshT = csb.tile([P, SH_TILES * P], dtype=bf16)
            nc.sync.dma_start(out=vocshT[:, : 6 * 512], in_=vocsh_d[:, : 6 * 512])
            nc.sync.dma_start(out=vocshT[:, 6 * 512 :], in_=vocsh_d[:, 6 * 512 :])
            vocsT = csb.tile([P, UNI], dtype=bf16)
            for h in range(4):
                nc.sync.dma_start(
                    out=vocsT[:, h * 4096 : (h + 1) * 4096],
                    in_=vocs_d[:, h * 4096 : (h + 1) * 4096],
                )

            def sweep_group(srcT, g4, do_s, s_acc, s_first, s_last, table_base, ntile):
                nt = min(4, ntile - g4 * 4)
                pl4 = p_m.tile([P, 512], dtype=f32, tag="m")
                for i in range(nt):
                    t4 = g4 * 4 + i
                    nc.tensor.matmul(
                        out=pl4[:, i * P : (i + 1) * P],
                        lhsT=srcT[:, t4 * P : (t4 + 1) * P], rhs=embTb[:],
                        start=True, stop=True, skip_group_check=True,
                    )
                ex4 = sbB.tile([P, 512], dtype=bf16, tag="ex4")
                nc.scalar.activation(
                    out=ex4[:, : nt * P], in_=pl4[:, : nt * P],
                    func=EXP, bias=negc0[:, :1],
                )
                if do_s:
                    nc.tensor.matmul(
                        out=s_acc[:1, : nt * P],
                        lhsT=onesb_col[:, :1],
                        rhs=ex4[:, : nt * P],
                        start=s_first, stop=s_last,
                        skip_group_check=True,
                    )
                else:
                    # one [512,128] write; row order = (p,i) inside the group,
                    # i.e. table position tau(u) = (u//512)*512 + (u%128)*4 + (u%512)//128
                    base = table_base + g4 * 512
                    nc.sync.dma_start(
                        out=table_d[base : base + nt * P, :],
                        in_=ex4[:, : nt * P],
                    )

            # shard sweep for s (49 tiles -> 13 groups)
            s_acc = p_cs.tile([1, 512], dtype=f32, tag="cs")
            for g4 in range(13):
                sweep_group(vocshT, g4, True, s_acc,
                            s_first=(g4 == 0), s_last=(g4 == 12),
                            table_base=0, ntile=SH_TILES)
            # compact table build (128 tiles -> 32 groups)
            for g4 in range(32):
                sweep_group(vocsT, g4, False, None, False, False,
                            table_base=0, ntile=UNI // P)

            # ---------- s: reduce, allreduce, postprocess ----------
            s4s = csb.tile([1, 512], dtype=f32)
            nc.scalar.copy(out=s4s[:], in_=s_acc[:1, :])
            s01 = csb.tile([1, P], dtype=f32)
            nc.vector.tensor_add(out=s01[:], in0=s4s[:1, 0:P], in1=s4s[:1, P : 2 * P])
            s23 = csb.tile([1, P], dtype=f32)
            nc.vector.tensor_add(
                out=s23[:], in0=s4s[:1, 2 * P : 3 * P], in1=s4s[:1, 3 * P : 4 * P]
            )
            s_row = csb.tile([1, P], dtype=f32)
            nc.vector.tensor_add(out=s_row[:], in0=s01[:], in1=s23[:])
            if USE_COLLECTIVE:
                nc.sync.dma_start(out=sP_d[:, :], in_=s_row[:1, :])
                nc.gpsimd.collective_compute(
                    "AllReduce",
                    mybir.AluOpType.add,
                    replica_groups=[list(range(8))],
                    ins=[sP_d[:, :]],
                    outs=[sG_d[:, :]],
                )
                sg = csb.tile([1, P], dtype=f32)
                nc.sync.dma_start(out=sg[:], in_=sG_d[:, :])
            else:
                sg = s_row
            rowVb = csb.tile([1, P], dtype=bf16)
            nc.vector.tensor_scalar_mul(out=rowVb[:], in0=sg[:], scalar1=1.0 / 128.0)
            nc.sync.dma_start(out=table_d[UNI : UNI + 1, :], in_=rowVb[:1, :])
            sT = p_t.tile([P, 1], dtype=f32, tag="pt")
            nc.tensor.transpose(out=sT[:, :1], in_=sg[:1, :], identity=ident[:1, :1])
            rS = csb.tile([P, 1], dtype=f32)
            nc.vector.reciprocal(out=rS[:], in_=sT[:, :1])

            # ---------- scan ----------
            import concourse.bass as _b

            pP = None
            step = 0
            for t in range(NTILE_G + 1):
                rows = P if t < NTILE_G else REM
                gt = sbS.tile([P, P], dtype=bf16, tag="gt")
                nc.gpsimd.indirect_dma_start(
                    out=gt[:rows, :],
                    out_offset=None,
                    in_=table_d[:, :],
                    in_offset=_b.IndirectOffsetOnAxis(ap=offs_sb[:rows, t : t + 1], axis=0),
                )
                tpg = p_t.tile([P, P], dtype=bf16, tag="pt")
                nc.tensor.transpose(
                    out=tpg[:, :rows], in_=gt[:rows, :], identity=identb[:rows, :rows]
                )
                tmp = sbS.tile([P, P], dtype=bf16, tag="tmp")
                nc.scalar.activation(
                    out=tmp[:, :rows], in_=tpg[:, :rows], func=COPY, scale=rS[:, :1]
                )
                cs1 = p_cs.tile([1, P], dtype=f32, tag="cs")
                nc.tensor.matmul(
                    out=cs1[:1, :rows], lhsT=onesb_col[:, :1], rhs=tmp[:, :rows],
                    start=True, stop=True, skip_group_check=True,
                )
                lncs = sbS.tile([1, P], dtype=f32, tag="lncs")
                nc.scalar.activation(out=lncs[:1, :rows], in_=cs1[:1, :rows], func=LN)
                nc.vector.tensor_add(
                    out=lnacc_sb[:1, :rows], in0=lnacc_sb[:1, :rows], in1=lncs[:1, :rows]
                )
                rcsf = sbS.tile([1, P], dtype=f32, tag="rcsf")
                nc.vector.reciprocal_approx_fast(out=rcsf[:1, :rows], in_=cs1[:1, :rows])
                rcs = sbS.tile([1, P], dtype=bf16, tag="rcs")
                nc.scalar.copy(out=rcs[:1, :rows], in_=rcsf[:1, :rows])
                # main bc + e for cols [0, rows) or [0, 120) on full tiles
                emain = rows - NROW if rows == P else rows
                bc = p_bc.tile([P, P], dtype=f32, tag="bc")
                nc.tensor.matmul(
                    out=bc[:, :emain], lhsT=ones128_row[:1, :], rhs=rcs[:1, :emain],
                    start=True, stop=True, skip_group_check=True,
                )
                eT = sbS.tile([P, P], dtype=bf16, tag="eT")
                nc.vector.tensor_mul(
                    out=eT[:, :emain], in0=tmp[:, :emain], in1=bc[:, :emain]
                )

                nsteps = rows // NROW
                for j in range(nsteps):
                    ecols = eT[:, j * NROW : (j + 1) * NROW]
                    src = q0[:] if step == 0 else pP[:]
                    if step == HALF:
                        qfin = csb.tile([P, NROW], dtype=f32)
                        nc.vector.tensor_mul(out=qfin[:], in0=src, in1=ecols)
                        nc.sync.dma_start(out=outp_d[:, :], in_=qfin[:])
                        break
                    qt = sbS.tile([P, NROW], dtype=bf16, tag="qt")
                    nc.vector.tensor_mul(out=qt[:], in0=src, in1=ecols)
                    if step % R == 2 and rows == P:
                        csq = p_cs.tile([1, NROW], dtype=f32, tag="cs")
                        nc.tensor.matmul(
                            out=csq[:], lhsT=onesb_col[:, :1], rhs=qt[:],
                            start=True, stop=True, skip_group_check=True,
                        )
                        lnq = sbS.tile([1, NROW], dtype=f32, tag="lnq")
                        nc.scalar.activation(out=lnq[:], in_=csq[:], func=LN)
                        nc.vector.tensor_add(out=accr_sb[:], in0=accr_sb[:], in1=lnq[:])
                        rq = sbS.tile([1, NROW], dtype=bf16, tag="rq")
                        with nc.allow_low_precision(reason="bf16 renorm; logged fp32"):
                            nc.vector.reciprocal(out=rq[:], in_=csq[:])
                        rch = sbS.tile([1, NROW], dtype=bf16, tag="rch")
                        nc.vector.tensor_mul(
                            out=rch[:], in0=rcs[:1, emain:rows], in1=rq[:]
                        )
                        nc.tensor.matmul(
                            out=bc[:, emain:rows], lhsT=ones128_row[:1, :],
                            rhs=rch[:1, :],
                            start=True, stop=True, skip_group_check=True,
                        )
                        nc.vector.tensor_mul(
                            out=eT[:, emain:rows], in0=tmp[:, emain:rows],
                            in1=bc[:, emain:rows],
                        )
                    pP = p_m.tile([P, NROW], dtype=f32, tag="m")
                    nc.tensor.matmul(
                        out=pP[:], lhsT=Wb[:], rhs=qt[:], start=True, stop=True
                    )
                    step += 1

            nc.sync.dma_start(out=accr_d[:1, :], in_=accr_sb[:1, :])
            nc.sync.dma_start(out=lnacc_d[:1, :], in_=lnacc_sb[:1, :])

    if not nc.is_finalized():
        nc.finalize()
    return nc


def _get_nc():
    if "nc" not in _CACHE:
        _CACHE["nc"] = _build_nc()
    return _CACHE["nc"]


def _make_in_maps(x, start_w, start_b, cluster_trans_w, emb_cluster_w, cluster_vocab_w):
    import ml_dtypes

    bf = ml_dtypes.bfloat16
    x = np.asarray(x).astype(np.int64)
    voc = np.asarray(cluster_vocab_w).astype(np.float32)
    tr = np.ascontiguousarray(
        np.asarray(cluster_trans_w)[:, 0].reshape(K, K).astype(np.float32)
    )
    emb = np.ascontiguousarray(np.asarray(emb_cluster_w).astype(np.float32))
    sw = np.ascontiguousarray(np.asarray(start_w).astype(np.float32).reshape(K, 1))
    sb = np.ascontiguousarray(np.asarray(start_b).astype(np.float32).reshape(K, 1))
    vocT = voc.T.astype(bf)  # [K, V] bf16, same rounding the device applied

    in_maps = []
    for c in range(8):
        g = c % 4
        rows = np.arange(g * NROW, (g + 1) * NROW)
        xc = x[rows, 0:HALF] if c < 4 else x[rows, HALF:T]  # this core's half
        uniq = np.unique(xc)
        nu = uniq.size
        assert nu <= UNI
        vocsT = np.zeros((K, UNI), bf)
        vocsT[:, :nu] = vocT[:, uniq]
        vocshT = np.zeros((K, SH_TILES * P), bf)
        vocshT[:, :SH_ROWS] = vocT[:, c * SH_ROWS : (c + 1) * SH_ROWS]
        u = np.searchsorted(uniq, xc)  # [8, HALF] compact positions
        lut = (u // 512) * 512 + (u % 128) * 4 + (u % 512) // 128  # stored row
        idx = np.empty((NSLOT, NROW), np.int64)
        if c < 4:
            idx[0, :] = UNI
            idx[1:, :] = lut[:, 0:HALF].T
        else:
            idx[0:HALF, :] = lut[:, ::-1].T
            idx[HALF, :] = UNI
        flat = idx.reshape(-1)
        padded = np.zeros(P * (NTILE_G + 1), np.int64)
        padded[: flat.size] = flat
        offs = np.ascontiguousarray(padded.reshape(NTILE_G + 1, P).T.astype(np.int32))

        dirf = np.full((K, 1), 1.0 if c < 4 else 0.0, np.float32)
        in_maps.append(
            {
                "tr": tr, "emb": emb,
                "vocs": np.ascontiguousarray(vocsT),
                "vocsh": np.ascontiguousarray(vocshT),
                "sw": sw, "sb": sb, "dirf": dirf, "offs": offs,
            }
        )
    return in_maps


def _combine(res):
    losses = np.empty(N, np.float64)
    for c in range(4):
        f = res[c]["outp"].astype(np.float64)
        b = res[c + 4]["outp"].astype(np.float64)
        af = res[c]["accr"][0].astype(np.float64)
        ab = res[c + 4]["accr"][0].astype(np.float64)
        ln = (res[c]["lnacc"][0] + res[c + 4]["lnacc"][0]).astype(np.float64)
        corr_r = ln.reshape(16, NROW).sum(axis=0)
        rows = np.arange(c * NROW, (c + 1) * NROW)
        dots = (f * b).sum(axis=0)
        losses[rows] = np.log(dots) + af + ab + corr_r - T * np.log(128.0)
    return np.float32(-losses.mean())


def kernel(x, start_w, start_b, cluster_trans_w, emb_cluster_w, cluster_vocab_w):
    from concourse.bass_utils import run_bass_kernel_spmd

    nc = _get_nc()
    in_maps = _make_in_maps(
        x, start_w, start_b, cluster_trans_w, emb_cluster_w, cluster_vocab_w
    )
    res = run_bass_kernel_spmd(nc, in_maps, list(range(8))).results
    return _combine(res)
